# revision 4
# baseline (speedup 1.0000x reference)
"""Self-contained Trainium2 Bass kernel for the 3-layer GAT problem.

Sharding: nodes split across 8 NeuronCores into balanced 128-dst blocks
(B=50 blocks/core, T_B 128-edge chunks per block); edges live with their
destination core. 5 SPMD launches with host reshard between them. All
per-edge data is host-pre-gathered into dense bf16 tables (no on-device
gathers); edge->slot scatter uses host-precomputed 0/1 bf16 masks.
"""
import numpy as np
import ml_dtypes
from contextlib import ExitStack

from concourse import bass, bacc, mybir, tile
from concourse.masks import make_identity
from concourse.bass_utils import run_bass_kernel_spmd

BF16 = ml_dtypes.bfloat16
F32 = mybir.dt.float32
BF = mybir.dt.bfloat16

H = 8
NUM_GRAPHS = 128
EDGE_DIM = 147
N = 50000
E = 200000
NCORES = 8
NODES_PER_CORE = N // NCORES          # 6250
B = 50                                # blocks per core (50*128 = 6400 >= 6250)
BP = B * 128                          # padded own nodes 6400
GRP = 5                               # blocks per group
NG = B // GRP                         # 10 groups
C_SHIFT = np.float32(20.0)


# ---------------------------------------------------------------------------
# host-side planning
# ---------------------------------------------------------------------------

def build_static_plan(edge_index, batch):
    src = np.asarray(edge_index[0], dtype=np.int64)
    dst = np.asarray(edge_index[1], dtype=np.int64)
    batch = np.asarray(batch, dtype=np.int64)
    deg = np.bincount(dst, minlength=N)

    plan = {"deg": deg}
    cores = []
    T_B_needed = 1
    for c in range(NCORES):
        lo, hi = c * NODES_PER_CORE, (c + 1) * NODES_PER_CORE
        own = np.arange(lo, hi)
        # --- balance nodes into B blocks by in-degree (LPT greedy + repair) ---
        order = np.argsort(-deg[own], kind="stable")
        blk_load = np.zeros(B, dtype=np.int64)
        blk_fill = np.zeros(B, dtype=np.int64)
        node_slot = np.full(BP, -1, dtype=np.int64)  # slot -> node id
        slot_of = np.full(N, -1, dtype=np.int64)
        for n_local in order:
            node = own[n_local]
            cand = np.where(blk_fill < 128)[0]
            b = cand[np.argmin(blk_load[cand])]
            s = b * 128 + blk_fill[b]
            blk_fill[b] += 1
            blk_load[b] += deg[node]
            node_slot[s] = node
            slot_of[node] = s
        # --- edges of this core, ordered by (dst slot, original idx) ---
        emask = (dst >= lo) & (dst < hi)
        e_ids = np.nonzero(emask)[0]
        e_slot = slot_of[dst[e_ids]]
        eorder = np.lexsort((e_ids, e_slot))
        e_ids = e_ids[eorder]
        e_slot = e_slot[eorder]
        e_src = src[e_ids]
        e_blk = e_slot // 128
        blk_counts = np.bincount(e_blk, minlength=B)
        T_B_needed = max(T_B_needed, int(np.ceil(blk_counts.max() / 128)))
        cores.append(dict(
            node_slot=node_slot, blk_counts=blk_counts,
            e_src=e_src, e_slot=e_slot, e_blk=e_blk, e_ids=e_ids,
        ))
    T_B = T_B_needed
    plan["T_B"] = T_B
    SLOTS = B * T_B
    plan["SLOTS"] = SLOTS
    plan["cores"] = cores

    for c, cc in enumerate(cores):
        ne = len(cc["e_src"])
        starts = np.zeros(B, dtype=np.int64)
        starts[1:] = np.cumsum(cc["blk_counts"])[:-1]
        pos = np.arange(ne) - starts[cc["e_blk"]]
        t = pos // 128
        p = pos % 128
        s = cc["e_blk"] * T_B + t               # chunk index
        cc["lane_p"] = p
        cc["lane_s"] = s
        # per-(lane, chunk) arrays
        src_ps = np.full((128, SLOTS), -1, dtype=np.int64)
        dst_ps = np.full((128, SLOTS), -1, dtype=np.int64)
        ea_ps = np.full((128, SLOTS), -1, dtype=np.int64)
        dstl_ps = np.full((128, SLOTS), -1, dtype=np.int64)
        src_ps[p, s] = cc["e_src"]
        dst_ps[p, s] = cc["node_slot"][cc["e_slot"]]
        ea_ps[p, s] = cc["e_ids"]
        dstl_ps[p, s] = cc["e_slot"] % 128
        cc["src_ps"] = src_ps
        cc["dst_ps"] = dst_ps
        cc["ea_ps"] = ea_ps
        cc["valid_ps"] = src_ps >= 0
        # scatter mask [128, SLOTS*128]
        M = np.zeros((128, SLOTS * 128), dtype=BF16)
        vp, vs = np.nonzero(cc["valid_ps"])
        M[vp, vs * 128 + dstl_ps[vp, vs]] = 1
        cc["mask"] = M
        # degree reciprocal per slot, [128, B]
        rcpd = np.zeros((128, B), dtype=np.float32)
        vs2 = cc["node_slot"] >= 0
        rr = np.zeros(BP, dtype=np.float32)
        rr[vs2] = 1.0 / np.maximum(deg[cc["node_slot"][vs2]], 1.0)
        cc["rcpdeg"] = rr.reshape(B, 128).T.copy()
        cc["valid_slot"] = vs2
        # pool mask [128, B*128]
        G = np.zeros((128, B * 128), dtype=BF16)
        sl = np.nonzero(vs2)[0]
        G[sl % 128, (sl // 128) * 128 + batch[cc["node_slot"][sl]]] = 1
        cc["gmask"] = G

    cnt = np.bincount(batch, minlength=NUM_GRAPHS).astype(np.float32)
    plan["rcp_cnt"] = (1.0 / np.maximum(cnt, 1.0)).astype(np.float32)
    return plan


def prep_weights(inp):
    w = {}
    Ve = np.zeros((24, EDGE_DIM), dtype=np.float32)
    for l, Cl in enumerate([64, 64, 32]):
        We = np.asarray(inp[f"We{l}"])          # [H*Cl, EDGE_DIM]
        ae = np.asarray(inp[f"ae{l}"])[0]       # [H, Cl]
        for h in range(H):
            Ve[8 * l + h] = ae[h] @ We[h * Cl:(h + 1) * Cl]
        W = np.asarray(inp[f"W{l}"])            # [H*Cl, cin]
        a_s = np.asarray(inp[f"as{l}"])[0]
        a_d = np.asarray(inp[f"ad{l}"])[0]
        us = np.zeros((H, W.shape[1]), dtype=np.float32)
        ud = np.zeros((H, W.shape[1]), dtype=np.float32)
        for h in range(H):
            us[h] = a_s[h] @ W[h * Cl:(h + 1) * Cl]
            ud[h] = a_d[h] @ W[h * Cl:(h + 1) * Cl]
        w[f"usud{l}T"] = np.concatenate([us, ud], 0).T.copy()  # [cin,16]
    VeT = Ve.T.copy()                            # [147, 24]
    w["VeTA"] = VeT[0:128].astype(BF16)
    w["VeTB"] = VeT[128:147].astype(BF16)
    w["usud0T"] = w["usud0T"].astype(BF16)       # [64, 16]
    W0 = np.asarray(inp["W0"])                   # [512, 64]
    W0hT = np.zeros((64, 512), dtype=np.float32)
    for h in range(H):
        W0hT[:, h * 64:(h + 1) * 64] = W0[h * 64:(h + 1) * 64, :].T
    W0d = np.zeros((512, 128), dtype=np.float32)  # stacked diag blocks
    for k in range(4):
        W0d[k * 128:k * 128 + 64, 0:64] = W0hT[:, (2 * k) * 64:(2 * k + 1) * 64]
        W0d[k * 128 + 64:k * 128 + 128, 64:128] = W0hT[:, (2 * k + 1) * 64:(2 * k + 2) * 64]
    w["W0d"] = W0d.astype(BF16)
    w["W1T"] = np.asarray(inp["W1"]).T.astype(BF16).copy()   # [512, 512]
    w["usud1T"] = w["usud1T"].astype(BF16)
    w["W2T"] = np.asarray(inp["W2"]).T.astype(BF16).copy()   # [512, 256]
    w["usud2T"] = w["usud2T"].astype(BF16)
    rep = lambda v: np.tile(np.asarray(v, dtype=np.float32)[None, :], (128, 1))
    w["b0row"] = rep(inp["b0"]); w["b1row"] = rep(inp["b1"]); w["b2row"] = rep(inp["b2"])
    w["negc1"] = rep(-w["W1T"].astype(np.float32).sum(0))
    w["negca1"] = rep(-w["usud1T"].astype(np.float32).sum(0))
    w["negc2"] = rep(-w["W2T"].astype(np.float32).sum(0))
    w["negca2"] = rep(-w["usud2T"].astype(np.float32).sum(0))
    w["WcT"] = np.asarray(inp["Wc"]).T.astype(np.float32).copy()   # [256, 32]
    w["bcrow"] = rep(inp["bc"])
    return w


def scatter_back(plan, shards, width, dtype):
    full = np.zeros((N, width), dtype=dtype)
    for c in range(NCORES):
        cc = plan["cores"][c]
        v = cc["valid_slot"]
        full[cc["node_slot"][v]] = np.asarray(shards[c])[v]
    return full


def build_T(plan, c, a_full, el_out_c, xp_full, XW, with_ex, lidx):
    cc = plan["cores"][c]
    SLOTS = plan["SLOTS"]
    REC = 24 + XW + (8 if with_ex else 0)
    T = np.zeros((128, SLOTS, REC), dtype=BF16)
    v = cc["valid_ps"]
    sidx = cc["src_ps"][v]
    didx = cc["dst_ps"][v]
    T[v, 0:8] = a_full[sidx, 0:8]
    T[v, 8:16] = a_full[didx, 8:16]
    el3 = np.asarray(el_out_c).reshape(128, SLOTS, 24)[:, :, lidx * 8:(lidx + 1) * 8]
    T[:, :, 16:24] = el3
    T[v, 24:24 + XW] = xp_full[sidx]
    return np.ascontiguousarray(T.reshape(128, SLOTS * REC))


def ell_slice(elloop_c, lidx):
    e = np.asarray(elloop_c).reshape(128, B, 24)[:, :, lidx * 8:(lidx + 1) * 8]
    return np.ascontiguousarray(e.reshape(128, B * 8))


# ---------------------------------------------------------------------------
# device kernels
# ---------------------------------------------------------------------------

def _ap(base, off, dims):
    a = base if isinstance(base, bass.AP) else base[:]
    return bass.AP(a.tensor, a.offset + off, [a.ap[0]] + dims)


def new_nc():
    return bacc.Bacc("TRN2", target_bir_lowering=False, debug=False, num_devices=8,
                     num_swdge_queues=4)


def build_launchA(T_B):
    """edge-attr projection el (all 3 layers) + self-loop el means + alpha0."""
    SLOTS = B * T_B
    S = SLOTS * 128
    CHB = GRP * T_B          # chunks per DMA batch (= GRP blocks)

    nc = new_nc()
    eaT = nc.dram_tensor("eaT", [EDGE_DIM, S], BF, kind="ExternalInput")
    M_t = nc.dram_tensor("M", [128, S], BF, kind="ExternalInput")
    VeTA_t = nc.dram_tensor("VeTA", [128, 24], BF, kind="ExternalInput")
    VeTB_t = nc.dram_tensor("VeTB", [19, 24], BF, kind="ExternalInput")
    rcpdeg_t = nc.dram_tensor("rcpdeg", [128, B], F32, kind="ExternalInput")
    ownxT_t = nc.dram_tensor("ownxT", [64, BP], BF, kind="ExternalInput")
    usud0T_t = nc.dram_tensor("usud0T", [64, 16], BF, kind="ExternalInput")
    el_out = nc.dram_tensor("el_out", [128, SLOTS * 24], BF, kind="ExternalOutput")
    elloop_out = nc.dram_tensor("elloop_out", [128, B * 24], F32, kind="ExternalOutput")
    a0_out = nc.dram_tensor("a0_out", [BP, 16], F32, kind="ExternalOutput")

    with tile.TileContext(nc) as tc:
        with ExitStack() as ctx:
            res = ctx.enter_context(tc.tile_pool(name="res", bufs=1))
            VeTA_sb = res.tile([128, 24], BF, tag="vea")
            nc.sync.dma_start(out=VeTA_sb[:], in_=VeTA_t[:, :])
            VeTB_sb = res.tile([19, 24], BF, tag="veb")
            nc.sync.dma_start(out=VeTB_sb[:], in_=VeTB_t[:, :])
            rcpdeg_sb = res.tile([128, B], F32, tag="rcpd")
            nc.sync.dma_start(out=rcpdeg_sb[:], in_=rcpdeg_t[:, :])
            usud0_sb = res.tile([64, 16], BF, tag="us0")
            nc.sync.dma_start(out=usud0_sb[:], in_=usud0T_t[:, :])
            ell_sb = res.tile([128, B * 24], F32, tag="ell")

            with tc.tile_pool(name="elp", bufs=2) as elp, \
                 tc.tile_pool(name="elps", bufs=4, space="PSUM") as elps, \
                 tc.tile_pool(name="ellps", bufs=2, space="PSUM") as ellps:
                for cb in range(SLOTS // CHB):
                    eaA = elp.tile([128, CHB * 128], BF, tag="eaA")
                    nc.sync.dma_start(
                        out=eaA[:],
                        in_=bass.AP(eaT[:, :].tensor, cb * CHB * 128,
                                    [[S, 128], [1, CHB * 128]]))
                    eaB = elp.tile([19, CHB * 128], BF, tag="eaB")
                    nc.sync.dma_start(
                        out=eaB[:],
                        in_=bass.AP(eaT[:, :].tensor, 128 * S + cb * CHB * 128,
                                    [[S, 19], [1, CHB * 128]]))
                    Mb = elp.tile([128, CHB * 128], BF, tag="Mb")
                    nc.sync.dma_start(out=Mb[:],
                                      in_=M_t[:, cb * CHB * 128:(cb + 1) * CHB * 128])
                    elbuf = elp.tile([128, CHB * 24], BF, tag="elbuf")
                    for ci in range(CHB):
                        c = cb * CHB + ci
                        el_ps = elps.tile([128, 24], F32, space="PSUM", tag="elps")
                        nc.tensor.matmul(out=el_ps[:], lhsT=eaA[:, ci * 128:(ci + 1) * 128],
                                         rhs=VeTA_sb[:], start=True, stop=False)
                        nc.tensor.matmul(out=el_ps[:], lhsT=eaB[0:19, ci * 128:(ci + 1) * 128],
                                         rhs=VeTB_sb[:], start=False, stop=True)
                        nc.scalar.copy(out=elbuf[:, ci * 24:(ci + 1) * 24], in_=el_ps[:])
                        t_in_b = c % T_B
                        if t_in_b == 0:
                            ell_ps = ellps.tile([128, 24], F32, space="PSUM", tag="ellps")
                        nc.tensor.matmul(out=ell_ps[:],
                                         lhsT=Mb[:, ci * 128:(ci + 1) * 128],
                                         rhs=elbuf[:, ci * 24:(ci + 1) * 24],
                                         start=(t_in_b == 0), stop=(t_in_b == T_B - 1))
                        if t_in_b == T_B - 1:
                            bidx = c // T_B
                            nc.vector.tensor_scalar_mul(
                                ell_sb[:, bidx * 24:(bidx + 1) * 24], ell_ps[:],
                                rcpdeg_sb[:, bidx:bidx + 1])
                    nc.sync.dma_start(
                        out=el_out[:, cb * CHB * 24:(cb + 1) * CHB * 24], in_=elbuf[:])
                nc.sync.dma_start(out=elloop_out[:, :], in_=ell_sb[:])

            # alpha0 = x_own @ usud0
            with tc.tile_pool(name="afp", bufs=2) as afp, \
                 tc.tile_pool(name="afps", bufs=4, space="PSUM") as afps:
                CB = 10
                for cb in range(0, B, CB):
                    n = min(CB, B - cb)
                    xt = afp.tile([64, CB * 128], BF, tag="xt")
                    nc.sync.dma_start(
                        out=xt[:, :n * 128],
                        in_=bass.AP(ownxT_t[:, :].tensor, cb * 128,
                                    [[BP, 64], [1, n * 128]]))
                    abuf = afp.tile([128, CB * 16], F32, tag="abuf")
                    for ci in range(n):
                        a_ps = afps.tile([128, 16], F32, space="PSUM", tag="aps")
                        nc.tensor.matmul(out=a_ps[:], lhsT=xt[:, ci * 128:(ci + 1) * 128],
                                         rhs=usud0_sb[:], start=True, stop=True)
                        nc.scalar.copy(out=abuf[:, ci * 16:(ci + 1) * 16], in_=a_ps[:])
                    nc.sync.dma_start(
                        out=bass.AP(a0_out[:, :].tensor, cb * 128 * 16,
                                    [[16, 128], [128 * 16, n], [1, 16]]),
                        in_=abuf[:, :n * 16].rearrange("p (c s) -> p c s", s=16))
    nc.compile()
    return nc


def build_attn(T_B, layer):
    """One GAT layer: softmax attention + scatter + (projection | pooling)."""
    XW = [512, 512, 256][layer]      # aggregation width
    CH = [64, 64, 32][layer]         # per-head width in agg space
    TXW = [64, 512, 256][layer]      # xp record cols in T table
    FW = [64, 512, 256][layer]       # ownF width
    with_ex = layer != 0
    REC = 24 + TXW + (8 if with_ex else 0)
    SLOTS = B * T_B
    S = SLOTS * 128
    GT = GRP * T_B

    nc = new_nc()
    T_t = nc.dram_tensor("T", [128, SLOTS * REC], BF, kind="ExternalInput")
    M_t = nc.dram_tensor("M", [128, S], BF, kind="ExternalInput")
    ownF_t = nc.dram_tensor("ownF", [BP, FW], BF, kind="ExternalInput")
    aown_t = nc.dram_tensor("aown", [BP, 16], F32, kind="ExternalInput")
    ell_t = nc.dram_tensor("ell", [128, B * 8], F32, kind="ExternalInput")
    brow_t = nc.dram_tensor("brow", [128, 512 if layer == 0 else XW], F32,
                            kind="ExternalInput")
    if layer == 0:
        W0d_t = nc.dram_tensor("W0d", [512, 128], BF, kind="ExternalInput")
        WT_t = nc.dram_tensor("WT", [512, 512], BF, kind="ExternalInput")
        usudT_t = nc.dram_tensor("usudT", [512, 16], BF, kind="ExternalInput")
        negc_t = nc.dram_tensor("negc", [128, 512], F32, kind="ExternalInput")
        negca_t = nc.dram_tensor("negca", [128, 16], F32, kind="ExternalInput")
        HCO = 512
        xp_out = nc.dram_tensor("xp_out", [BP, HCO], BF, kind="ExternalOutput")
        a_out = nc.dram_tensor("a_out", [BP, 16], F32, kind="ExternalOutput")
    elif layer == 1:
        WT_t = nc.dram_tensor("WT", [512, 256], BF, kind="ExternalInput")
        usudT_t = nc.dram_tensor("usudT", [512, 16], BF, kind="ExternalInput")
        negc_t = nc.dram_tensor("negc", [128, 256], F32, kind="ExternalInput")
        negca_t = nc.dram_tensor("negca", [128, 16], F32, kind="ExternalInput")
        HCO = 256
        xp_out = nc.dram_tensor("xp_out", [BP, HCO], BF, kind="ExternalOutput")
        a_out = nc.dram_tensor("a_out", [BP, 16], F32, kind="ExternalOutput")
    else:
        G_t = nc.dram_tensor("G", [128, B * 128], BF, kind="ExternalInput")
        pool_out = nc.dram_tensor("pool_out", [128, 256], F32, kind="ExternalOutput")

    with tile.TileContext(nc) as tc:
        with ExitStack() as ctx:
            res = ctx.enter_context(tc.tile_pool(name="res", bufs=1))
            cshift = res.tile([128, 1], F32, tag="cshift")
            nc.any.memset(cshift[:], -C_SHIFT)
            aown_sb = res.tile([128, B * 16], F32, tag="aown")
            nc.sync.dma_start(
                out=aown_sb[:],
                in_=bass.AP(aown_t[:, :].tensor, 0,
                            [[16, 128], [16 * 128, B], [1, 16]]))
            ell_sb = res.tile([128, B * 8], F32, tag="ellr")
            nc.sync.dma_start(out=ell_sb[:], in_=ell_t[:, :])
            brow_sb = res.tile([128, 512 if layer == 0 else XW], F32, tag="brow")
            nc.sync.dma_start(out=brow_sb[:], in_=brow_t[:, :])
            if layer != 2:
                ident = res.tile([128, 128], BF, tag="ident")
                make_identity(nc, ident[:])
                WT_sb = [res.tile([128, HCO], BF, tag=f"wt{k}", name=f"wt{k}")
                         for k in range(4)]
                usud_sb = [res.tile([128, 16], BF, tag=f"us{k}", name=f"us{k}")
                           for k in range(4)]
                for k in range(4):
                    nc.sync.dma_start(out=WT_sb[k][:], in_=WT_t[k * 128:(k + 1) * 128, :])
                    nc.sync.dma_start(out=usud_sb[k][:], in_=usudT_t[k * 128:(k + 1) * 128, :])
                negc_sb = res.tile([128, HCO], F32, tag="negc")
                nc.sync.dma_start(out=negc_sb[:], in_=negc_t[:, :])
                negca_sb = res.tile([128, 16], F32, tag="negca")
                nc.sync.dma_start(out=negca_sb[:], in_=negca_t[:, :])
            if layer == 0:
                W0d_sb = [res.tile([128, 128], BF, tag=f"w0d{k}", name=f"w0d{k}")
                          for k in range(4)]
                for k in range(4):
                    nc.sync.dma_start(out=W0d_sb[k][:], in_=W0d_t[k * 128:(k + 1) * 128, :])
            if layer == 2:
                G_sb = res.tile([128, B * 128], BF, tag="G")
                nc.sync.dma_start(out=G_sb[:], in_=G_t[:, :])

            ld = ctx.enter_context(tc.tile_pool(name="ld", bufs=2))
            sml = ctx.enter_context(tc.tile_pool(name="sml", bufs=3))
            ps_agg = ctx.enter_context(tc.tile_pool(name="psagg", bufs=2, space="PSUM"))
            ps_den = ctx.enter_context(tc.tile_pool(name="psden", bufs=1, space="PSUM"))
            if layer != 2:
                ps_tp = ctx.enter_context(tc.tile_pool(name="pstp", bufs=2, space="PSUM"))
                ps_xp = ctx.enter_context(tc.tile_pool(name="psxp", bufs=1, space="PSUM"))
                ps_a = ctx.enter_context(tc.tile_pool(name="psa", bufs=1, space="PSUM"))
            if layer == 0:
                ps_h1 = ctx.enter_context(tc.tile_pool(name="psh1", bufs=1, space="PSUM"))
            if layer == 2:
                ps_pool = ctx.enter_context(tc.tile_pool(name="pspool", bufs=1, space="PSUM"))
                pool_ps = ps_pool.tile([128, 256], F32, space="PSUM", tag="pool")

            TT = mybir.AluOpType
            for g in range(NG):
                g0 = g * GRP
                Tg = ld.tile([128, GT * REC], BF, tag="Tg")
                nc.sync.dma_start(out=Tg[:],
                                  in_=T_t[:, g0 * T_B * REC:(g0 + GRP) * T_B * REC])
                Mg = ld.tile([128, GT * 128], BF, tag="Mg")
                nc.sync.dma_start(out=Mg[:],
                                  in_=M_t[:, g0 * T_B * 128:(g0 + GRP) * T_B * 128])
                owng = ld.tile([128, GRP * FW], BF, tag="owng")
                nc.sync.dma_start(
                    out=owng[:],
                    in_=bass.AP(ownF_t[:, :].tensor, g0 * 128 * FW,
                                [[FW, 128], [128 * FW, GRP], [1, FW]]))
                # ---- logits for the whole group ----
                zsum = sml.tile([128, GT * 8], F32, tag="zsum")
                nc.vector.tensor_tensor(out=zsum[:],
                                        in0=_ap(Tg, 0, [[REC, GT], [1, 8]]),
                                        in1=_ap(Tg, 8, [[REC, GT], [1, 8]]),
                                        op=TT.add)
                nc.vector.tensor_tensor(out=zsum[:], in0=zsum[:],
                                        in1=_ap(Tg, 16, [[REC, GT], [1, 8]]),
                                        op=TT.add)
                zt = sml.tile([128, GT * 8], F32, tag="zt")
                nc.vector.tensor_scalar_mul(zt[:], zsum[:], 0.2)
                nc.vector.tensor_tensor(out=zsum[:], in0=zsum[:], in1=zt[:],
                                        op=TT.max)
                if layer == 0:
                    v1g = ld.tile([128, GT * 520], BF, tag="v1g")
                    nc.scalar.activation(
                        _ap(v1g, 512, [[520, GT], [1, 8]]), zsum[:],
                        mybir.ActivationFunctionType.Exp, bias=cshift[:], scale=1.0)
                    nc.vector.tensor_tensor(
                        out=_ap(v1g, 0, [[520, GT], [64, 8], [1, 64]]),
                        in0=_ap(Tg, 24, [[REC, GT], [0, 8], [1, 64]]),
                        in1=_ap(v1g, 512, [[520, GT], [1, 8], [0, 64]]),
                        op=TT.mult)
                    vw = 520
                    vbase = v1g
                    voff = 0
                else:
                    nc.scalar.activation(
                        _ap(Tg, 24 + TXW, [[REC, GT], [1, 8]]), zsum[:],
                        mybir.ActivationFunctionType.Exp, bias=cshift[:], scale=1.0)
                    nc.vector.tensor_tensor(
                        out=_ap(Tg, 24, [[REC, GT], [CH, 8], [1, CH]]),
                        in0=_ap(Tg, 24, [[REC, GT], [CH, 8], [1, CH]]),
                        in1=_ap(Tg, 24 + TXW, [[REC, GT], [1, 8], [0, CH]]),
                        op=TT.mult)
                    vw = REC
                    vbase = Tg
                    voff = 24
                # ---- self-loop for the whole group ----
                zs = sml.tile([128, GRP * 8], F32, tag="zs")
                nc.vector.tensor_tensor(
                    out=zs[:],
                    in0=_ap(aown_sb, g0 * 16, [[16, GRP], [1, 8]]),
                    in1=_ap(aown_sb, g0 * 16 + 8, [[16, GRP], [1, 8]]), op=TT.add)
                nc.vector.tensor_tensor(
                    out=zs[:], in0=zs[:],
                    in1=_ap(ell_sb, g0 * 8, [[8, GRP], [1, 8]]), op=TT.add)
                zst = sml.tile([128, GRP * 8], F32, tag="zst")
                nc.vector.tensor_scalar_mul(zst[:], zs[:], 0.2)
                nc.vector.tensor_tensor(out=zs[:], in0=zs[:], in1=zst[:], op=TT.max)
                exs_g = sml.tile([128, GRP * 8], F32, tag="exsg")
                nc.scalar.activation(exs_g[:], zs[:],
                                     mybir.ActivationFunctionType.Exp,
                                     bias=cshift[:], scale=1.0)

                for bg in range(GRP):
                    b = g0 + bg
                    agg_ps = ps_agg.tile([128, XW], F32, space="PSUM", tag="agg")
                    den_ps = ps_den.tile([128, 8], F32, space="PSUM", tag="den")
                    for t in range(T_B):
                        s = bg * T_B + t
                        lhsT = _ap(Mg, s * 128, [[1, 128]])
                        nc.tensor.matmul(out=agg_ps[:], lhsT=lhsT,
                                         rhs=_ap(vbase, s * vw + voff, [[1, XW]]),
                                         start=(t == 0), stop=(t == T_B - 1))
                        nc.tensor.matmul(out=den_ps[:], lhsT=lhsT,
                                         rhs=_ap(vbase, s * vw + voff + XW, [[1, 8]]),
                                         start=(t == 0), stop=(t == T_B - 1))
                    den = sml.tile([128, 8], F32, tag="dent")
                    nc.vector.tensor_tensor(out=den[:], in0=den_ps[:],
                                            in1=exs_g[:, bg * 8:(bg + 1) * 8], op=TT.add)
                    rcp = sml.tile([128, 8], F32, tag="rcp")
                    nc.vector.reciprocal(rcp[:], den[:])
                    asf = sml.tile([128, 8], F32, tag="asf")
                    nc.vector.tensor_tensor(out=asf[:], in0=exs_g[:, bg * 8:(bg + 1) * 8],
                                            in1=rcp[:], op=TT.mult)
                    selft = sml.tile([128, XW], BF, tag="selft")
                    if layer == 0:
                        own_ap = _ap(owng, bg * 64, [[0, 8], [1, 64]])
                    else:
                        own_ap = _ap(owng, bg * FW, [[CH, 8], [1, CH]])
                    nc.vector.tensor_tensor(
                        out=_ap(selft, 0, [[CH, 8], [1, CH]]),
                        in0=own_ap,
                        in1=_ap(asf, 0, [[1, 8], [0, CH]]), op=TT.mult)
                    hsb = sml.tile([128, XW], BF, tag="hsb")
                    nc.vector.tensor_tensor(
                        out=_ap(hsb, 0, [[CH, 8], [1, CH]]),
                        in0=_ap(agg_ps, 0, [[CH, 8], [1, CH]]),
                        in1=_ap(rcp, 0, [[1, 8], [0, CH]]), op=TT.mult)
                    nc.vector.tensor_tensor(out=hsb[:], in0=hsb[:], in1=selft[:],
                                            op=TT.add)
                    if layer == 2:
                        nc.vector.tensor_tensor(out=hsb[:], in0=hsb[:],
                                                in1=brow_sb[:, :XW], op=TT.add)
                        nc.tensor.matmul(out=pool_ps[:],
                                         lhsT=_ap(G_sb, b * 128, [[1, 128]]),
                                         rhs=hsb[:], start=(b == 0), stop=(b == B - 1))
                        continue
                    if layer == 0:
                        h1_ps = ps_h1.tile([128, 512], F32, space="PSUM", tag="h1")
                        for k in range(4):
                            tp = ps_tp.tile([128, 128], BF, space="PSUM", tag="tp")
                            nc.tensor.transpose(out=tp[:], in_=hsb[:, k * 128:(k + 1) * 128],
                                                identity=ident[:])
                            hT = sml.tile([128, 128], BF, tag="hT")
                            nc.scalar.copy(out=hT[:], in_=tp[:])
                            nc.tensor.matmul(out=h1_ps[:, k * 128:(k + 1) * 128],
                                             lhsT=hT[:], rhs=W0d_sb[k][:],
                                             start=True, stop=True)
                        hb = sml.tile([128, 512], F32, tag="hb")
                        nc.vector.tensor_tensor(out=hb[:], in0=h1_ps[:],
                                                in1=brow_sb[:], op=TT.add)
                        esrc = hb
                    else:
                        nc.vector.tensor_tensor(out=hsb[:], in0=hsb[:],
                                                in1=brow_sb[:, :XW], op=TT.add)
                        esrc = hsb
                    mm = sml.tile([128, 512], BF, tag="mmt")
                    nc.vector.tensor_scalar_min(mm[:], esrc[:], 0.0)
                    ee = sml.tile([128, 512], BF, tag="ee")
                    nc.scalar.activation(ee[:], mm[:],
                                         mybir.ActivationFunctionType.Exp,
                                         bias=0.0, scale=1.0)
                    rr = sml.tile([128, 512], BF, tag="rr")
                    nc.scalar.activation(rr[:], esrc[:],
                                         mybir.ActivationFunctionType.Relu,
                                         bias=0.0, scale=1.0)
                    g2 = sml.tile([128, 512], BF, tag="g2")
                    nc.vector.tensor_tensor(out=g2[:], in0=rr[:], in1=ee[:], op=TT.add)
                    xp_ps = ps_xp.tile([128, HCO], F32, space="PSUM", tag="xp")
                    a_ps = ps_a.tile([128, 16], F32, space="PSUM", tag="a")
                    for k in range(4):
                        tp = ps_tp.tile([128, 128], BF, space="PSUM", tag="tp")
                        nc.tensor.transpose(out=tp[:], in_=g2[:, k * 128:(k + 1) * 128],
                                            identity=ident[:])
                        gT = sml.tile([128, 128], BF, tag="gT")
                        nc.scalar.copy(out=gT[:], in_=tp[:])
                        nc.tensor.matmul(out=xp_ps[:], lhsT=gT[:], rhs=WT_sb[k][:],
                                         start=(k == 0), stop=(k == 3))
                        nc.tensor.matmul(out=a_ps[:], lhsT=gT[:], rhs=usud_sb[k][:],
                                         start=(k == 0), stop=(k == 3))
                    xp_sb = sml.tile([128, HCO], BF, tag="xpsb")
                    nc.vector.tensor_tensor(out=xp_sb[:], in0=xp_ps[:],
                                            in1=negc_sb[:], op=TT.add)
                    nc.sync.dma_start(out=xp_out[b * 128:(b + 1) * 128, :], in_=xp_sb[:])
                    a_sb = sml.tile([128, 16], F32, tag="asb")
                    nc.vector.tensor_tensor(out=a_sb[:], in0=a_ps[:],
                                            in1=negca_sb[:], op=TT.add)
                    nc.sync.dma_start(out=a_out[b * 128:(b + 1) * 128, :], in_=a_sb[:])

            if layer == 2:
                pool_sb = res.tile([128, 256], F32, tag="poolsb")
                nc.vector.tensor_copy(out=pool_sb[:], in_=pool_ps[:])
                nc.sync.dma_start(out=pool_out[:, :], in_=pool_sb[:])
    nc.compile()
    return nc


def build_launchE():
    nc = new_nc()
    pp = nc.dram_tensor("pp", [8 * 128, 256], F32, kind="ExternalInput")
    rcpc = nc.dram_tensor("rcpc", [128, 1], F32, kind="ExternalInput")
    WcT = nc.dram_tensor("WcT", [256, 32], F32, kind="ExternalInput")
    bcrow = nc.dram_tensor("bcrow", [128, 32], F32, kind="ExternalInput")
    out = nc.dram_tensor("out", [128, 32], F32, kind="ExternalOutput")
    with tile.TileContext(nc) as tc:
        with ExitStack() as ctx:
            res = ctx.enter_context(tc.tile_pool(name="res", bufs=1))
            pool = ctx.enter_context(tc.tile_pool(name="p", bufs=2))
            ps_tp = ctx.enter_context(tc.tile_pool(name="pstp", bufs=2, space="PSUM"))
            ps_o = ctx.enter_context(tc.tile_pool(name="pso", bufs=1, space="PSUM"))
            acc = res.tile([128, 256], F32, tag="acc")
            nc.sync.dma_start(out=acc[:], in_=pp[0:128, :])
            for c in range(1, 8):
                t = pool.tile([128, 256], F32, tag="t", name=f"t{c}")
                nc.sync.dma_start(out=t[:], in_=pp[c * 128:(c + 1) * 128, :])
                nc.vector.tensor_tensor(out=acc[:], in0=acc[:], in1=t[:],
                                        op=mybir.AluOpType.add)
            rc = res.tile([128, 1], F32, tag="rc")
            nc.sync.dma_start(out=rc[:], in_=rcpc[:, :])
            nc.vector.tensor_scalar_mul(acc[:], acc[:], rc[:])
            ident = res.tile([128, 128], F32, tag="id")
            make_identity(nc, ident[:])
            wc_sb = [res.tile([128, 32], F32, tag=f"wc{k}", name=f"wc{k}") for k in range(2)]
            for k in range(2):
                nc.sync.dma_start(out=wc_sb[k][:], in_=WcT[k * 128:(k + 1) * 128, :])
            bc_sb = res.tile([128, 32], F32, tag="bc")
            nc.sync.dma_start(out=bc_sb[:], in_=bcrow[:, :])
            o_ps = ps_o.tile([128, 32], F32, space="PSUM", tag="o")
            for k in range(2):
                tp = ps_tp.tile([128, 128], F32, space="PSUM", tag="tp", name=f"tp{k}")
                nc.tensor.transpose(out=tp[:], in_=acc[:, k * 128:(k + 1) * 128],
                                    identity=ident[:])
                tps = pool.tile([128, 128], F32, tag="tps", name=f"tps{k}")
                nc.vector.tensor_copy(out=tps[:], in_=tp[:])
                nc.tensor.matmul(out=o_ps[:], lhsT=tps[:], rhs=wc_sb[k][:],
                                 start=(k == 0), stop=(k == 1))
            osb = res.tile([128, 32], F32, tag="osb")
            nc.vector.tensor_tensor(out=osb[:], in0=o_ps[:], in1=bc_sb[:],
                                    op=mybir.AluOpType.add)
            nc.sync.dma_start(out=out[:, :], in_=osb[:])
    nc.compile()
    return nc


# ---------------------------------------------------------------------------
# driver
# ---------------------------------------------------------------------------

_NC_CACHE = {}
PROFILE = False
LAST_EXEC_NS = []


def _get_ncs(T_B):
    key = (T_B,)
    if key not in _NC_CACHE:
        _NC_CACHE[key] = (build_launchA(T_B),
                          build_attn(T_B, 0),
                          build_attn(T_B, 1),
                          build_attn(T_B, 2),
                          build_launchE())
    return _NC_CACHE[key]


def _run(nc, in_maps):
    res = run_bass_kernel_spmd(nc, in_maps, core_ids=list(range(8)),
                               trace=PROFILE)
    if PROFILE:
        LAST_EXEC_NS.append(res.exec_time_ns)
    return res


def kernel(**inputs):
    inp = {k: np.asarray(v) for k, v in inputs.items()}
    plan = build_static_plan(inp["edge_index"], inp["batch"])
    w = prep_weights(inp)
    T_B = plan["T_B"]
    SLOTS = plan["SLOTS"]
    S = SLOTS * 128
    ncA, nc0, nc1, nc2, ncE = _get_ncs(T_B)
    LAST_EXEC_NS.clear()

    x = np.asarray(inp["x"], dtype=np.float32)
    ea = np.asarray(inp["edge_attr"], dtype=np.float32)
    x_bf = x.astype(BF16)

    # ---- launch A: el + alpha0 ----
    in_maps = []
    for c in range(NCORES):
        cc = plan["cores"][c]
        eaT = np.zeros((EDGE_DIM, S), dtype=BF16)
        v = cc["valid_ps"]
        vp, vs = np.nonzero(v)
        eaT[:, vs * 128 + vp] = ea[cc["ea_ps"][vp, vs]].T
        ownx = np.zeros((BP, 64), dtype=BF16)
        vs2 = cc["valid_slot"]
        ownx[vs2] = x_bf[cc["node_slot"][vs2]]
        cc["ownx"] = ownx
        in_maps.append(dict(
            eaT=eaT, M=cc["mask"], VeTA=w["VeTA"], VeTB=w["VeTB"],
            rcpdeg=cc["rcpdeg"], ownxT=np.ascontiguousarray(ownx.T),
            usud0T=w["usud0T"]))
    rA = _run(ncA, in_maps)
    a0_full = scatter_back(plan, [rA.results[c]["a0_out"] for c in range(NCORES)],
                           16, np.float32)

    # ---- launch 0 ----
    in_maps = []
    for c in range(NCORES):
        cc = plan["cores"][c]
        T0 = build_T(plan, c, a0_full, rA.results[c]["el_out"], x_bf,
                     64, False, 0)
        in_maps.append(dict(
            T=T0, M=cc["mask"], ownF=cc["ownx"], aown=rA.results[c]["a0_out"],
            ell=ell_slice(rA.results[c]["elloop_out"], 0), brow=w["b0row"],
            W0d=w["W0d"], WT=w["W1T"], usudT=w["usud1T"],
            negc=w["negc1"], negca=w["negca1"]))
    r0 = _run(nc0, in_maps)
    xp1_full = scatter_back(plan, [r0.results[c]["xp_out"] for c in range(NCORES)],
                            512, BF16)
    a1_full = scatter_back(plan, [r0.results[c]["a_out"] for c in range(NCORES)],
                           16, np.float32)

    # ---- launch 1 ----
    in_maps = []
    for c in range(NCORES):
        cc = plan["cores"][c]
        T1 = build_T(plan, c, a1_full, rA.results[c]["el_out"], xp1_full,
                     512, True, 1)
        in_maps.append(dict(
            T=T1, M=cc["mask"], ownF=r0.results[c]["xp_out"],
            aown=r0.results[c]["a_out"],
            ell=ell_slice(rA.results[c]["elloop_out"], 1), brow=w["b1row"],
            WT=w["W2T"], usudT=w["usud2T"],
            negc=w["negc2"], negca=w["negca2"]))
    r1 = _run(nc1, in_maps)
    xp2_full = scatter_back(plan, [r1.results[c]["xp_out"] for c in range(NCORES)],
                            256, BF16)
    a2_full = scatter_back(plan, [r1.results[c]["a_out"] for c in range(NCORES)],
                           16, np.float32)

    # ---- launch 2 (+ pool partials) ----
    in_maps = []
    for c in range(NCORES):
        cc = plan["cores"][c]
        T2 = build_T(plan, c, a2_full, rA.results[c]["el_out"], xp2_full,
                     256, True, 2)
        in_maps.append(dict(
            T=T2, M=cc["mask"], ownF=r1.results[c]["xp_out"],
            aown=r1.results[c]["a_out"],
            ell=ell_slice(rA.results[c]["elloop_out"], 2), brow=w["b2row"],
            G=cc["gmask"]))
    r2 = _run(nc2, in_maps)

    # ---- launch E (combine + final linear) ----
    pp = np.concatenate([np.asarray(r2.results[c]["pool_out"], dtype=np.float32)
                         for c in range(NCORES)], 0)
    in_maps = [dict(pp=pp, rcpc=plan["rcp_cnt"][:, None].astype(np.float32),
                    WcT=w["WcT"], bcrow=w["bcrow"])] * NCORES
    rE = _run(ncE, in_maps)
    return np.asarray(rE.results[0]["out"], dtype=np.float32)


# revision 17
# speedup vs baseline: 1.7036x; 1.7036x over previous
"""Self-contained Trainium2 Bass kernel for the 3-layer GAT problem.

Sharding: nodes split across 8 NeuronCores into balanced 128-dst blocks
(B=50 blocks/core); edges live with their destination core. 5 SPMD
launches with host reshard between them. All per-edge data is host-pre-
gathered into dense bf16 tables (no on-device gathers); edge->slot
scatter uses host-precomputed 0/1 bf16 masks; self-loops are folded in
as a dedicated identity-mask chunk per block.
"""
import numpy as np
import ml_dtypes
from contextlib import ExitStack

from concourse import bass, bacc, mybir, tile
from concourse.masks import make_identity
from concourse.bass_utils import run_bass_kernel_spmd

BF16 = ml_dtypes.bfloat16
F32 = mybir.dt.float32
BF = mybir.dt.bfloat16

H = 8
NUM_GRAPHS = 128
EDGE_DIM = 147
N = 50000
E = 200000
NCORES = 8
NODES_PER_CORE = N // NCORES          # 6250
B = 50                                # blocks per core (50*128 = 6400 >= 6250)
BP = B * 128                          # padded own nodes 6400
GRP = 5                               # blocks per group
NG = B // GRP                         # 10 groups
C_SHIFT = np.float32(20.0)


# ---------------------------------------------------------------------------
# host-side planning
# ---------------------------------------------------------------------------

def build_static_plan(edge_index, batch):
    src = np.asarray(edge_index[0], dtype=np.int64)
    dst = np.asarray(edge_index[1], dtype=np.int64)
    batch = np.asarray(batch, dtype=np.int64)
    deg = np.bincount(dst, minlength=N)

    plan = {"deg": deg}
    cores = []
    T_B_needed = 1
    for c in range(NCORES):
        lo, hi = c * NODES_PER_CORE, (c + 1) * NODES_PER_CORE
        own = np.arange(lo, hi)
        # --- balance nodes into B blocks by in-degree (LPT greedy) ---
        order = np.argsort(-deg[own], kind="stable")
        blk_load = np.zeros(B, dtype=np.int64)
        blk_fill = np.zeros(B, dtype=np.int64)
        node_slot = np.full(BP, -1, dtype=np.int64)  # slot -> node id
        slot_of = np.full(N, -1, dtype=np.int64)
        for n_local in order:
            node = own[n_local]
            cand = np.where(blk_fill < 128)[0]
            b = cand[np.argmin(blk_load[cand])]
            s = b * 128 + blk_fill[b]
            blk_fill[b] += 1
            blk_load[b] += deg[node]
            node_slot[s] = node
            slot_of[node] = s
        # --- edges of this core, ordered by (dst slot, original idx) ---
        emask = (dst >= lo) & (dst < hi)
        e_ids = np.nonzero(emask)[0]
        e_slot = slot_of[dst[e_ids]]
        eorder = np.lexsort((e_ids, e_slot))
        e_ids = e_ids[eorder]
        e_slot = e_slot[eorder]
        e_src = src[e_ids]
        e_blk = e_slot // 128
        blk_counts = np.bincount(e_blk, minlength=B)
        T_B_needed = max(T_B_needed, int(np.ceil(blk_counts.max() / 128)))
        cores.append(dict(
            node_slot=node_slot, blk_counts=blk_counts,
            e_src=e_src, e_slot=e_slot, e_blk=e_blk, e_ids=e_ids,
        ))
    T_B = T_B_needed                 # real-edge chunks per block
    T_BE = T_B + 1                   # + self-loop chunk
    plan["T_B"] = T_B
    SLOTS_R = B * T_B
    SLOTS_E = B * T_BE
    plan["SLOTS_R"] = SLOTS_R
    plan["SLOTS_E"] = SLOTS_E
    plan["cores"] = cores

    for c, cc in enumerate(cores):
        ne = len(cc["e_src"])
        starts = np.zeros(B, dtype=np.int64)
        starts[1:] = np.cumsum(cc["blk_counts"])[:-1]
        pos = np.arange(ne) - starts[cc["e_blk"]]
        t = pos // 128
        p = pos % 128
        sr = cc["e_blk"] * T_B + t              # launch-A chunk index
        se = cc["e_blk"] * T_BE + t             # attention chunk index
        dstl = cc["e_slot"] % 128
        # launch-A per-(lane, chunk) edge-attr row mapping
        ea_ps = np.full((128, SLOTS_R), -1, dtype=np.int64)
        ea_ps[p, sr] = cc["e_ids"]
        cc["ea_ps"] = ea_ps
        # attention-table per-(lane, chunk) arrays incl. self chunk
        src_ps = np.full((128, SLOTS_E), -1, dtype=np.int64)
        dst_ps = np.full((128, SLOTS_E), -1, dtype=np.int64)
        src_ps[p, se] = cc["e_src"]
        dst_ps[p, se] = cc["node_slot"][cc["e_slot"]]
        selfcols = np.arange(B) * T_BE + T_B
        ns2 = cc["node_slot"].reshape(B, 128).T
        src_ps[:, selfcols] = ns2
        dst_ps[:, selfcols] = ns2
        cc["src_ps"] = src_ps
        cc["dst_ps"] = dst_ps
        cc["valid_ps"] = src_ps >= 0
        # launch-A scatter mask (real edges only)
        Mr = np.zeros((128, SLOTS_R * 128), dtype=BF16)
        Mr[p, sr * 128 + dstl] = 1
        cc["mask_r"] = Mr
        # attention scatter mask (real edges + full identity self chunk)
        M = np.zeros((128, SLOTS_E * 128), dtype=BF16)
        M[p, se * 128 + dstl] = 1
        lanes = np.arange(128)
        for b in range(B):
            M[lanes, (b * T_BE + T_B) * 128 + lanes] = 1
        cc["mask"] = M
        MT = np.zeros((128, SLOTS_E * 128), dtype=FP8)
        MT[dstl, se * 128 + p] = 1
        for b in range(B):
            MT[lanes, (b * T_BE + T_B) * 128 + lanes] = 1
        cc["maskT"] = MT
        # degree reciprocal per slot, [128, B]
        vs2 = cc["node_slot"] >= 0
        rr = np.zeros(BP, dtype=np.float32)
        rr[vs2] = 1.0 / np.maximum(deg[cc["node_slot"][vs2]], 1.0)
        cc["rcpdeg"] = rr.reshape(B, 128).T.copy()
        cc["valid_slot"] = vs2
        # pool mask [128, B*128]
        G = np.zeros((128, B * 128), dtype=BF16)
        sl = np.nonzero(vs2)[0]
        G[sl % 128, (sl // 128) * 128 + batch[cc["node_slot"][sl]]] = 1
        cc["gmask"] = G

    cnt = np.bincount(batch, minlength=NUM_GRAPHS).astype(np.float32)
    plan["rcp_cnt"] = (1.0 / np.maximum(cnt, 1.0)).astype(np.float32)
    return plan


def perm_il(XW, CH):
    """old-index array: new j=c*8+h -> old h*CH+c."""
    idx = np.empty(XW, dtype=np.int64)
    for cc_ in range(CH):
        for hh in range(8):
            idx[cc_ * 8 + hh] = hh * CH + cc_
    return idx


PERM64 = perm_il(512, 64)
PERM32 = perm_il(256, 32)


def prep_weights(inp):
    w = {}
    Ve = np.zeros((24, EDGE_DIM), dtype=np.float32)
    for l, Cl in enumerate([64, 64, 32]):
        We = np.asarray(inp[f"We{l}"])          # [H*Cl, EDGE_DIM]
        ae = np.asarray(inp[f"ae{l}"])[0]       # [H, Cl]
        for h in range(H):
            Ve[8 * l + h] = ae[h] @ We[h * Cl:(h + 1) * Cl]
        W = np.asarray(inp[f"W{l}"])            # [H*Cl, cin]
        a_s = np.asarray(inp[f"as{l}"])[0]
        a_d = np.asarray(inp[f"ad{l}"])[0]
        us = np.zeros((H, W.shape[1]), dtype=np.float32)
        ud = np.zeros((H, W.shape[1]), dtype=np.float32)
        for h in range(H):
            us[h] = a_s[h] @ W[h * Cl:(h + 1) * Cl]
            ud[h] = a_d[h] @ W[h * Cl:(h + 1) * Cl]
        w[f"usud{l}T"] = np.concatenate([us, ud], 0).T.copy()  # [cin,16]
    VeT = Ve.T.copy()                            # [147, 24]
    w["VeTA"] = VeT[0:128].astype(BF16)
    w["VeTB"] = VeT[128:147].astype(BF16)
    w["usud0T"] = w["usud0T"].astype(BF16)       # [64, 16]
    W0 = np.asarray(inp["W0"])                   # [512, 64]
    W0hT = np.zeros((64, 512), dtype=np.float32)
    for h in range(H):
        W0hT[:, h * 64:(h + 1) * 64] = W0[h * 64:(h + 1) * 64, :].T
    w["W0hT"] = W0hT.astype(BF16)
    w["W1T"] = np.ascontiguousarray(
        np.asarray(inp["W1"]).T.astype(BF16)[PERM64])        # [512, 512] row-perm
    w["usud1T"] = np.ascontiguousarray(w["usud1T"].astype(BF16)[PERM64])
    W2T = np.asarray(inp["W2"]).T.astype(np.float32)         # [512, 256]
    usud2T = w["usud2T"].astype(np.float32)
    w["Wc1"] = np.ascontiguousarray(
        np.concatenate([W2T, usud2T], 1)[PERM64].astype(BF16))
    rep = lambda v: np.tile(np.asarray(v, dtype=np.float32)[None, :], (128, 1))
    w["b0row"] = rep(np.asarray(inp["b0"], dtype=np.float32)[PERM64])
    w["b1row"] = rep(np.asarray(inp["b1"], dtype=np.float32)[PERM64])
    w["b2row"] = rep(np.asarray(inp["b2"], dtype=np.float32)[PERM32])
    w["negc1"] = rep(-w["W1T"].astype(np.float32).sum(0))
    w["negca1"] = rep(-w["usud1T"].astype(np.float32).sum(0))
    w["negc2c"] = rep(-w["Wc1"].astype(np.float32).sum(0))   # [128, 272]
    w["WcT"] = np.ascontiguousarray(
        np.asarray(inp["Wc"]).T.astype(np.float32)[PERM32])  # [256, 32] row-perm
    w["bcrow"] = rep(inp["bc"])
    return w


def scatter_back(plan, shards, width, dtype):
    full = np.zeros((N, width), dtype=dtype)
    for c in range(NCORES):
        cc = plan["cores"][c]
        v = cc["valid_slot"]
        full[cc["node_slot"][v]] = np.asarray(shards[c])[v]
    return full


def build_T(plan, c, a_full, el_out_c, elloop_c, xp_full, XW, lidx):
    cc = plan["cores"][c]
    T_B = plan["T_B"]
    T_BE = T_B + 1
    SLOTS_E = plan["SLOTS_E"]
    REC = 32 + XW
    T = np.zeros((128, SLOTS_E, REC), dtype=BF16)
    v = cc["valid_ps"]
    sidx = cc["src_ps"][v]
    didx = cc["dst_ps"][v]
    T[v, 0:8] = a_full[sidx, 0:8]
    T[v, 8:16] = a_full[didx, 8:16]
    el4 = np.asarray(el_out_c).reshape(128, B, T_B, 24)[:, :, :, lidx * 8:(lidx + 1) * 8]
    Tv = T.reshape(128, B, T_BE, REC)
    Tv[:, :, :T_B, 16:24] = el4
    ell = np.asarray(elloop_c).reshape(128, B, 24)[:, :, lidx * 8:(lidx + 1) * 8]
    Tv[:, :, T_B, 16:24] = ell
    perm = PERM64 if XW == 512 else PERM32
    T[v, 24:24 + XW] = xp_full[:, perm][sidx]
    return np.ascontiguousarray(T.reshape(128, SLOTS_E * REC))


# ---------------------------------------------------------------------------
# device kernels
# ---------------------------------------------------------------------------

def _ap(base, off, dims):
    a = base if isinstance(base, bass.AP) else base[:]
    return bass.AP(a.tensor, a.offset + off, [a.ap[0]] + dims)


def new_nc():
    return bacc.Bacc("TRN2", target_bir_lowering=False, debug=False, num_devices=8,
                     num_swdge_queues=4)


def build_launchA(T_B):
    """el projection (all 3 layers) + self-loop el means + alpha0 + xp0."""
    SLOTS_R = B * T_B
    S = SLOTS_R * 128
    CHB = GRP * T_B          # chunks per DMA batch (= GRP blocks)

    nc = new_nc()
    eaT = nc.dram_tensor("eaT", [EDGE_DIM, S], BF, kind="ExternalInput")
    M_t = nc.dram_tensor("M", [128, S], BF, kind="ExternalInput")
    VeTA_t = nc.dram_tensor("VeTA", [128, 24], BF, kind="ExternalInput")
    VeTB_t = nc.dram_tensor("VeTB", [19, 24], BF, kind="ExternalInput")
    rcpdeg_t = nc.dram_tensor("rcpdeg", [128, B], F32, kind="ExternalInput")
    ownxT_t = nc.dram_tensor("ownxT", [64, BP], BF, kind="ExternalInput")
    usud0T_t = nc.dram_tensor("usud0T", [64, 16], BF, kind="ExternalInput")
    W0hT_t = nc.dram_tensor("W0hT", [64, 512], BF, kind="ExternalInput")
    el_out = nc.dram_tensor("el_out", [128, SLOTS_R * 24], BF, kind="ExternalOutput")
    elloop_out = nc.dram_tensor("elloop_out", [128, B * 24], F32, kind="ExternalOutput")
    a0_out = nc.dram_tensor("a0_out", [BP, 16], BF, kind="ExternalOutput")
    xp0_out = nc.dram_tensor("xp0_out", [BP, 512], BF, kind="ExternalOutput")

    with tile.TileContext(nc) as tc:
        with ExitStack() as ctx:
            res = ctx.enter_context(tc.tile_pool(name="res", bufs=1))
            VeTA_sb = res.tile([128, 24], BF, tag="vea")
            nc.sync.dma_start(out=VeTA_sb[:], in_=VeTA_t[:, :])
            VeTB_sb = res.tile([19, 24], BF, tag="veb")
            nc.sync.dma_start(out=VeTB_sb[:], in_=VeTB_t[:, :])
            rcpdeg_sb = res.tile([128, B], F32, tag="rcpd")
            nc.sync.dma_start(out=rcpdeg_sb[:], in_=rcpdeg_t[:, :])
            usud0_sb = res.tile([64, 16], BF, tag="us0")
            nc.sync.dma_start(out=usud0_sb[:], in_=usud0T_t[:, :])
            W0hT_sb = res.tile([64, 512], BF, tag="w0h")
            nc.sync.dma_start(out=W0hT_sb[:], in_=W0hT_t[:, :])
            ell_sb = res.tile([128, B * 24], F32, tag="ell")

            with tc.tile_pool(name="elp", bufs=2) as elp, \
                 tc.tile_pool(name="elps", bufs=4, space="PSUM") as elps, \
                 tc.tile_pool(name="ellps", bufs=2, space="PSUM") as ellps:
                for cb in range(SLOTS_R // CHB):
                    eaA = elp.tile([128, CHB * 128], BF, tag="eaA")
                    nc.sync.dma_start(
                        out=eaA[:],
                        in_=bass.AP(eaT[:, :].tensor, cb * CHB * 128,
                                    [[S, 128], [1, CHB * 128]]))
                    eaB = elp.tile([19, CHB * 128], BF, tag="eaB")
                    nc.sync.dma_start(
                        out=eaB[:],
                        in_=bass.AP(eaT[:, :].tensor, 128 * S + cb * CHB * 128,
                                    [[S, 19], [1, CHB * 128]]))
                    Mb = elp.tile([128, CHB * 128], BF, tag="Mb")
                    nc.sync.dma_start(out=Mb[:],
                                      in_=M_t[:, cb * CHB * 128:(cb + 1) * CHB * 128])
                    elbuf = elp.tile([128, CHB * 24], BF, tag="elbuf")
                    for ci in range(CHB):
                        c = cb * CHB + ci
                        el_ps = elps.tile([128, 24], F32, space="PSUM", tag="elps")
                        nc.tensor.matmul(out=el_ps[:], lhsT=eaA[:, ci * 128:(ci + 1) * 128],
                                         rhs=VeTA_sb[:], start=True, stop=False)
                        nc.tensor.matmul(out=el_ps[:], lhsT=eaB[0:19, ci * 128:(ci + 1) * 128],
                                         rhs=VeTB_sb[:], start=False, stop=True)
                        nc.scalar.copy(out=elbuf[:, ci * 24:(ci + 1) * 24], in_=el_ps[:])
                        t_in_b = c % T_B
                        if t_in_b == 0:
                            ell_ps = ellps.tile([128, 24], F32, space="PSUM", tag="ellps")
                        nc.tensor.matmul(out=ell_ps[:],
                                         lhsT=Mb[:, ci * 128:(ci + 1) * 128],
                                         rhs=elbuf[:, ci * 24:(ci + 1) * 24],
                                         start=(t_in_b == 0), stop=(t_in_b == T_B - 1))
                        if t_in_b == T_B - 1:
                            bidx = c // T_B
                            nc.vector.tensor_scalar_mul(
                                ell_sb[:, bidx * 24:(bidx + 1) * 24], ell_ps[:],
                                rcpdeg_sb[:, bidx:bidx + 1])
                    nc.sync.dma_start(
                        out=el_out[:, cb * CHB * 24:(cb + 1) * CHB * 24], in_=elbuf[:])
                nc.sync.dma_start(out=elloop_out[:, :], in_=ell_sb[:])

            # alpha0 = x_own @ usud0 ; xp0 = x_own @ W0 (per head)
            with tc.tile_pool(name="afp", bufs=2) as afp, \
                 tc.tile_pool(name="afps", bufs=2, space="PSUM") as afps, \
                 tc.tile_pool(name="xfps", bufs=2, space="PSUM") as xfps:
                CB = 10
                for cb in range(0, B, CB):
                    n = min(CB, B - cb)
                    xt = afp.tile([64, CB * 128], BF, tag="xt")
                    nc.sync.dma_start(
                        out=xt[:, :n * 128],
                        in_=bass.AP(ownxT_t[:, :].tensor, cb * 128,
                                    [[BP, 64], [1, n * 128]]))
                    abuf = afp.tile([128, CB * 16], BF, tag="abuf")
                    xbuf = afp.tile([128, CB * 512], BF, tag="xbuf")
                    for ci in range(n):
                        a_ps = afps.tile([128, 16], F32, space="PSUM", tag="aps")
                        nc.tensor.matmul(out=a_ps[:], lhsT=xt[:, ci * 128:(ci + 1) * 128],
                                         rhs=usud0_sb[:], start=True, stop=True)
                        nc.scalar.copy(out=abuf[:, ci * 16:(ci + 1) * 16], in_=a_ps[:])
                        x_ps = xfps.tile([128, 512], F32, space="PSUM", tag="xps")
                        nc.tensor.matmul(out=x_ps[:], lhsT=xt[:, ci * 128:(ci + 1) * 128],
                                         rhs=W0hT_sb[:], start=True, stop=True)
                        nc.scalar.copy(out=xbuf[:, ci * 512:(ci + 1) * 512], in_=x_ps[:])
                    nc.sync.dma_start(
                        out=bass.AP(a0_out[:, :].tensor, cb * 128 * 16,
                                    [[16, 128], [128 * 16, n], [1, 16]]),
                        in_=abuf[:, :n * 16].rearrange("p (c s) -> p c s", s=16))
                    nc.sync.dma_start(
                        out=bass.AP(xp0_out[:, :].tensor, cb * 128 * 512,
                                    [[512, 128], [128 * 512, n], [1, 512]]),
                        in_=xbuf[:, :n * 512].rearrange("p (c s) -> p c s", s=512))
    nc.compile()
    return nc


def build_attn(T_B, layer, with_bias=False):
    """One GAT layer: softmax attention + scatter + (projection | pooling).

    Software-pipelined emission: per-block weighted-feature multiply and
    aggregation matmuls are issued one block ahead of the block tail so no
    engine head-of-line blocks on a cross-engine dependency.
    """
    T_BE = T_B + 1
    XW = [512, 512, 256][layer]      # aggregation width
    CH = [64, 64, 32][layer]         # per-head width in agg space
    REC = 32 + XW
    SLOTS_E = B * T_BE
    S = SLOTS_E * 128
    GT = GRP * T_BE

    nc = new_nc()
    T_t = nc.dram_tensor("T", [128, SLOTS_E * REC], BF, kind="ExternalInput")
    M_t = nc.dram_tensor("M", [128, S], F8, kind="ExternalInput")
    if with_bias:
        brow_t = nc.dram_tensor("brow", [128, XW], BF, kind="ExternalInput")
    if layer == 0:
        WT_t = nc.dram_tensor("WT", [512, 512], BF, kind="ExternalInput")
        usudT_t = nc.dram_tensor("usudT", [512, 16], BF, kind="ExternalInput")
        negc_t = nc.dram_tensor("negc", [128, 512], F32, kind="ExternalInput")
        negca_t = nc.dram_tensor("negca", [128, 16], F32, kind="ExternalInput")
        HCO = 512
        xp_out = nc.dram_tensor("xp_out", [BP, HCO], BF, kind="ExternalOutput")
        a_out = nc.dram_tensor("a_out", [BP, 16], BF, kind="ExternalOutput")
    elif layer == 1:
        WT_t = nc.dram_tensor("WT", [512, 272], BF, kind="ExternalInput")
        negc_t = nc.dram_tensor("negc", [128, 272], F32, kind="ExternalInput")
        HCO = 272
        xp_out = nc.dram_tensor("xp_out", [BP, 256], BF, kind="ExternalOutput")
        a_out = nc.dram_tensor("a_out", [BP, 16], BF, kind="ExternalOutput")
    else:
        G_t = nc.dram_tensor("G", [128, B * 128], F8, kind="ExternalInput")
        pool_out = nc.dram_tensor("pool_out", [128, 256], F32, kind="ExternalOutput")

    with tile.TileContext(nc) as tc:
        with ExitStack() as ctx:
            res = ctx.enter_context(tc.tile_pool(name="res", bufs=1))
            cshift = res.tile([128, 1], F32, tag="cshift")
            nc.any.memset(cshift[:], -C_SHIFT)
            if with_bias:
                brow_sb = res.tile([128, XW], BF, tag="brow")
                nc.sync.dma_start(out=brow_sb[:], in_=brow_t[:, :])
            if layer != 2:
                ident = res.tile([128, 128], BF, tag="ident")
                make_identity(nc, ident[:])
                WT_sb = [res.tile([128, HCO], BF, tag=f"wt{k}", name=f"wt{k}")
                         for k in range(4)]
                for k in range(4):
                    nc.sync.dma_start(out=WT_sb[k][:], in_=WT_t[k * 128:(k + 1) * 128, :])
                negc_sb = res.tile([128, HCO], F32, tag="negc")
                nc.sync.dma_start(out=negc_sb[:], in_=negc_t[:, :])
            if layer == 0:
                usud_sb = [res.tile([128, 16], BF, tag=f"us{k}", name=f"us{k}")
                           for k in range(4)]
                for k in range(4):
                    nc.sync.dma_start(out=usud_sb[k][:], in_=usudT_t[k * 128:(k + 1) * 128, :])
                negca_sb = res.tile([128, 16], F32, tag="negca")
                nc.sync.dma_start(out=negca_sb[:], in_=negca_t[:, :])
            if layer == 2:
                G_sb = res.tile([128, B * 128], F8, tag="G")
                nc.sync.dma_start(out=G_sb[:], in_=G_t[:, :])

            ld = ctx.enter_context(tc.tile_pool(name="ld", bufs=3))
            sml = ctx.enter_context(tc.tile_pool(name="sml", bufs=4))
            nagg = 3 if layer != 2 else 4
            ps_agg = ctx.enter_context(tc.tile_pool(name="psagg", bufs=nagg, space="PSUM"))
            if layer != 2:
                ps_den = ctx.enter_context(tc.tile_pool(name="psden", bufs=1, space="PSUM"))
                ps_tp = ctx.enter_context(tc.tile_pool(name="pstp", bufs=2, space="PSUM"))
                ps_xp = ctx.enter_context(tc.tile_pool(name="psxp", bufs=1, space="PSUM"))
            if layer == 0:
                ps_a = ctx.enter_context(tc.tile_pool(name="psa", bufs=1, space="PSUM"))
            if layer == 2:
                ps_pool = ctx.enter_context(tc.tile_pool(name="pspool", bufs=1, space="PSUM"))
                pool_ps = ps_pool.tile([128, 256], F32, space="PSUM", tag="pool")

            TT = mybir.AluOpType
            tiles = {}
            psum = {}

            def emit_load(g):
                g0 = g * GRP
                Tg = ld.tile([128, GT * REC], BF, tag="Tg", name=f"Tg{g}")
                nc.sync.dma_start(out=Tg[:],
                                  in_=T_t[:, g0 * T_BE * REC:(g0 + GRP) * T_BE * REC])
                Mg = ld.tile([128, GT * 128], F8, tag="Mg", name=f"Mg{g}")
                nc.sync.dma_start(out=Mg[:],
                                  in_=M_t[:, g0 * T_BE * 128:(g0 + GRP) * T_BE * 128])
                tiles[g] = (Tg, Mg)

            def emit_preproc(g):
                Tg, Mg = tiles[g]
                zsum = sml.tile([128, GT * 8], F32, tag="zsum", name=f"zs{g}")
                nc.vector.tensor_tensor(out=zsum[:],
                                        in0=_ap(Tg, 0, [[REC, GT], [1, 8]]),
                                        in1=_ap(Tg, 8, [[REC, GT], [1, 8]]),
                                        op=TT.add)
                nc.vector.tensor_tensor(out=zsum[:], in0=zsum[:],
                                        in1=_ap(Tg, 16, [[REC, GT], [1, 8]]),
                                        op=TT.add)
                nc.vector.scalar_tensor_tensor(
                    out=zsum[:], in0=zsum[:], scalar=0.2, in1=zsum[:],
                    op0=TT.mult, op1=TT.max)
                nc.scalar.activation(
                    _ap(Tg, 24 + XW, [[REC, GT], [1, 8]]), zsum[:],
                    mybir.ActivationFunctionType.Exp, bias=cshift[:], scale=1.0)

            def emit_agg(b):
                g, bg = divmod(b, GRP)
                Tg, Mg = tiles[g]
                nc.vector.tensor_tensor(
                    out=_ap(Tg, bg * T_BE * REC + 24, [[REC, T_BE], [8, CH], [1, 8]]),
                    in0=_ap(Tg, bg * T_BE * REC + 24, [[REC, T_BE], [8, CH], [1, 8]]),
                    in1=_ap(Tg, bg * T_BE * REC + 24 + XW, [[REC, T_BE], [0, CH], [1, 8]]),
                    op=TT.mult)
                if layer == 2:
                    agg_ps = ps_agg.tile([128, XW + 8], F32, space="PSUM",
                                         tag="agg", name=f"agg{b}")
                    for t in range(T_BE):
                        sl = bg * T_BE + t
                        nc.tensor.matmul(out=agg_ps[:],
                                         lhsT=_ap(Mg, sl * 128, [[1, 128]]),
                                         rhs=_ap(Tg, sl * REC + 24, [[1, XW + 8]]),
                                         start=(t == 0), stop=(t == T_BE - 1))
                    psum[b] = (agg_ps, None)
                else:
                    agg_ps = ps_agg.tile([128, XW], F32, space="PSUM",
                                         tag="agg", name=f"agg{b}")
                    den_ps = ps_den.tile([128, 8], F32, space="PSUM",
                                         tag="den", name=f"den{b}")
                    for t in range(T_BE):
                        sl = bg * T_BE + t
                        lhsT = _ap(Mg, sl * 128, [[1, 128]])
                        nc.tensor.matmul(out=agg_ps[:], lhsT=lhsT,
                                         rhs=_ap(Tg, sl * REC + 24, [[1, XW]]),
                                         start=(t == 0), stop=(t == T_BE - 1))
                        nc.tensor.matmul(out=den_ps[:], lhsT=lhsT,
                                         rhs=_ap(Tg, sl * REC + 24 + XW, [[1, 8]]),
                                         start=(t == 0), stop=(t == T_BE - 1))
                    psum[b] = (agg_ps, den_ps)

            def emit_tail(b):
                agg_ps, den_ps = psum.pop(b)
                den_ap = agg_ps[:, XW:XW + 8] if layer == 2 else den_ps[:]
                rcp = sml.tile([128, 8], F32, tag="rcp")
                nc.vector.reciprocal(rcp[:], den_ap)
                hsb = sml.tile([128, XW], BF, tag="hsb")
                nc.vector.tensor_tensor(
                    out=_ap(hsb, 0, [[8, CH], [1, 8]]),
                    in0=_ap(agg_ps, 0, [[8, CH], [1, 8]]),
                    in1=_ap(rcp, 0, [[0, CH], [1, 8]]), op=TT.mult)
                if with_bias:
                    nc.vector.tensor_tensor(out=hsb[:], in0=hsb[:],
                                            in1=brow_sb[:], op=TT.add)
                if layer == 2:
                    nc.tensor.matmul(out=pool_ps[:],
                                     lhsT=_ap(G_sb, b * 128, [[1, 128]]),
                                     rhs=hsb[:], start=(b == 0), stop=(b == B - 1))
                    return
                # elu'(h) = relu(h) + exp(min(h, 0)); min(h,0) = -relu(-h)
                mm = sml.tile([128, 512], BF, tag="mmt")
                nc.scalar.activation(mm[:], hsb[:],
                                     mybir.ActivationFunctionType.Relu,
                                     bias=0.0, scale=-1.0)
                ee = sml.tile([128, 512], BF, tag="ee")
                nc.scalar.activation(ee[:], mm[:],
                                     mybir.ActivationFunctionType.Exp,
                                     bias=0.0, scale=-1.0)
                g2 = sml.tile([128, 512], BF, tag="g2")
                nc.vector.scalar_tensor_tensor(
                    out=g2[:], in0=hsb[:], scalar=0.0, in1=ee[:],
                    op0=TT.max, op1=TT.add)
                xp_ps = ps_xp.tile([128, HCO], F32, space="PSUM", tag="xp")
                if layer == 0:
                    a_ps = ps_a.tile([128, 16], F32, space="PSUM", tag="a")
                for k in range(4):
                    tp = ps_tp.tile([128, 128], BF, space="PSUM", tag="tp")
                    nc.tensor.transpose(out=tp[:], in_=g2[:, k * 128:(k + 1) * 128],
                                        identity=ident[:])
                    gT = sml.tile([128, 128], BF, tag="gT")
                    nc.scalar.copy(out=gT[:], in_=tp[:])
                    nc.tensor.matmul(out=xp_ps[:], lhsT=gT[:], rhs=WT_sb[k][:],
                                     start=(k == 0), stop=(k == 3))
                    if layer == 0:
                        nc.tensor.matmul(out=a_ps[:], lhsT=gT[:], rhs=usud_sb[k][:],
                                         start=(k == 0), stop=(k == 3))
                xp_sb = sml.tile([128, HCO], BF, tag="xpsb")
                nc.vector.tensor_tensor(out=xp_sb[:], in0=xp_ps[:],
                                        in1=negc_sb[:], op=TT.add)
                if layer == 0:
                    nc.sync.dma_start(out=xp_out[b * 128:(b + 1) * 128, :],
                                      in_=xp_sb[:])
                    a_sb = sml.tile([128, 16], BF, tag="asb")
                    nc.vector.tensor_tensor(out=a_sb[:], in0=a_ps[:],
                                            in1=negca_sb[:], op=TT.add)
                    nc.sync.dma_start(out=a_out[b * 128:(b + 1) * 128, :], in_=a_sb[:])
                else:
                    nc.sync.dma_start(out=xp_out[b * 128:(b + 1) * 128, :],
                                      in_=xp_sb[:, 0:256])
                    nc.sync.dma_start(out=a_out[b * 128:(b + 1) * 128, :],
                                      in_=xp_sb[:, 256:272])

            emit_load(0)
            emit_preproc(0)
            if NG > 1:
                emit_load(1)
            for b in range(B + 1):
                if b < B:
                    g, bg = divmod(b, GRP)
                    if bg == 0 and g + 1 < NG:
                        emit_preproc(g + 1)
                        if g + 2 < NG:
                            emit_load(g + 2)
                    emit_agg(b)
                if b >= 1:
                    emit_tail(b - 1)

            if layer == 2:
                pool_sb = res.tile([128, 256], F32, tag="poolsb")
                nc.vector.tensor_copy(out=pool_sb[:], in_=pool_ps[:])
                nc.sync.dma_start(out=pool_out[:, :], in_=pool_sb[:])
    nc.compile()
    return nc


def build_launchE():
    nc = new_nc()
    pp = nc.dram_tensor("pp", [8 * 128, 256], F32, kind="ExternalInput")
    rcpc = nc.dram_tensor("rcpc", [128, 1], F32, kind="ExternalInput")
    WcT = nc.dram_tensor("WcT", [256, 32], F32, kind="ExternalInput")
    bcrow = nc.dram_tensor("bcrow", [128, 32], F32, kind="ExternalInput")
    out = nc.dram_tensor("out", [128, 32], F32, kind="ExternalOutput")
    with tile.TileContext(nc) as tc:
        with ExitStack() as ctx:
            res = ctx.enter_context(tc.tile_pool(name="res", bufs=1))
            pool = ctx.enter_context(tc.tile_pool(name="p", bufs=2))
            ps_tp = ctx.enter_context(tc.tile_pool(name="pstp", bufs=2, space="PSUM"))
            ps_o = ctx.enter_context(tc.tile_pool(name="pso", bufs=1, space="PSUM"))
            acc = res.tile([128, 256], F32, tag="acc")
            nc.sync.dma_start(out=acc[:], in_=pp[0:128, :])
            for c in range(1, 8):
                t = pool.tile([128, 256], F32, tag="t", name=f"t{c}")
                nc.sync.dma_start(out=t[:], in_=pp[c * 128:(c + 1) * 128, :])
                nc.vector.tensor_tensor(out=acc[:], in0=acc[:], in1=t[:],
                                        op=mybir.AluOpType.add)
            rc = res.tile([128, 1], F32, tag="rc")
            nc.sync.dma_start(out=rc[:], in_=rcpc[:, :])
            nc.vector.tensor_scalar_mul(acc[:], acc[:], rc[:])
            ident = res.tile([128, 128], F32, tag="id")
            make_identity(nc, ident[:])
            wc_sb = [res.tile([128, 32], F32, tag=f"wc{k}", name=f"wc{k}") for k in range(2)]
            for k in range(2):
                nc.sync.dma_start(out=wc_sb[k][:], in_=WcT[k * 128:(k + 1) * 128, :])
            bc_sb = res.tile([128, 32], F32, tag="bc")
            nc.sync.dma_start(out=bc_sb[:], in_=bcrow[:, :])
            o_ps = ps_o.tile([128, 32], F32, space="PSUM", tag="o")
            for k in range(2):
                tp = ps_tp.tile([128, 128], F32, space="PSUM", tag="tp", name=f"tp{k}")
                nc.tensor.transpose(out=tp[:], in_=acc[:, k * 128:(k + 1) * 128],
                                    identity=ident[:])
                tps = pool.tile([128, 128], F32, tag="tps", name=f"tps{k}")
                nc.vector.tensor_copy(out=tps[:], in_=tp[:])
                nc.tensor.matmul(out=o_ps[:], lhsT=tps[:], rhs=wc_sb[k][:],
                                 start=(k == 0), stop=(k == 1))
            osb = res.tile([128, 32], F32, tag="osb")
            nc.vector.tensor_tensor(out=osb[:], in0=o_ps[:], in1=bc_sb[:],
                                    op=mybir.AluOpType.add)
            nc.sync.dma_start(out=out[:, :], in_=osb[:])
    nc.compile()
    return nc


# ---------------------------------------------------------------------------
# driver
# ---------------------------------------------------------------------------

_NC_CACHE = {}
WITH_BIAS = [False, False, False]
PROFILE = False
LAST_EXEC_NS = []


def _get_ncs(T_B):
    key = T_B
    if key not in _NC_CACHE:
        _NC_CACHE[key] = (build_launchA(T_B),) + tuple(
            build_attn(T_B, l, with_bias=WITH_BIAS[l]) for l in range(3)
        ) + (build_launchE(),)
    return _NC_CACHE[key]


def _run(nc, in_maps):
    res = run_bass_kernel_spmd(nc, in_maps, core_ids=list(range(8)),
                               trace=PROFILE)
    if PROFILE:
        LAST_EXEC_NS.append(res.exec_time_ns)
    return res


def kernel(**inputs):
    inp = {k: np.asarray(v) for k, v in inputs.items()}
    plan = build_static_plan(inp["edge_index"], inp["batch"])
    w = prep_weights(inp)
    for l in range(3):
        WITH_BIAS[l] = bool(np.any(np.asarray(inp[f"b{l}"])))
    T_B = plan["T_B"]
    SLOTS_R = plan["SLOTS_R"]
    S_R = SLOTS_R * 128
    ncA, nc0, nc1, nc2, ncE = _get_ncs(T_B)
    LAST_EXEC_NS.clear()

    x = np.asarray(inp["x"], dtype=np.float32)
    ea = np.asarray(inp["edge_attr"], dtype=np.float32)
    x_bf = x.astype(BF16)

    # ---- launch A: el + alpha0 + xp0 ----
    in_maps = []
    for c in range(NCORES):
        cc = plan["cores"][c]
        eaT = np.zeros((EDGE_DIM, S_R), dtype=BF16)
        vp, vs = np.nonzero(cc["ea_ps"] >= 0)
        eaT[:, vs * 128 + vp] = ea[cc["ea_ps"][vp, vs]].T
        ownx = np.zeros((BP, 64), dtype=BF16)
        vs2 = cc["valid_slot"]
        ownx[vs2] = x_bf[cc["node_slot"][vs2]]
        in_maps.append(dict(
            eaT=eaT, M=cc["mask_r"], VeTA=w["VeTA"], VeTB=w["VeTB"],
            rcpdeg=cc["rcpdeg"], ownxT=np.ascontiguousarray(ownx.T),
            usud0T=w["usud0T"], W0hT=w["W0hT"]))
    rA = _run(ncA, in_maps)
    a0_full = scatter_back(plan, [rA.results[c]["a0_out"] for c in range(NCORES)],
                           16, BF16)
    xp0_full = scatter_back(plan, [rA.results[c]["xp0_out"] for c in range(NCORES)],
                            512, BF16)

    # ---- launch 0 ----
    in_maps = []
    for c in range(NCORES):
        cc = plan["cores"][c]
        T0 = build_T(plan, c, a0_full, rA.results[c]["el_out"],
                     rA.results[c]["elloop_out"], xp0_full, 512, 0)
        im = dict(
            T=T0, M=cc["mask"],
            WT=w["W1T"], usudT=w["usud1T"],
            negc=w["negc1"], negca=w["negca1"])
        if WITH_BIAS[0]:
            im["brow"] = w["b0row"].astype(BF16)
        in_maps.append(im)
    r0 = _run(nc0, in_maps)
    xp1_full = scatter_back(plan, [r0.results[c]["xp_out"] for c in range(NCORES)],
                            512, BF16)
    a1_full = scatter_back(plan, [r0.results[c]["a_out"] for c in range(NCORES)],
                           16, BF16)

    # ---- launch 1 ----
    in_maps = []
    for c in range(NCORES):
        cc = plan["cores"][c]
        T1 = build_T(plan, c, a1_full, rA.results[c]["el_out"],
                     rA.results[c]["elloop_out"], xp1_full, 512, 1)
        im = dict(T=T1, M=cc["mask"], WT=w["Wc1"], negc=w["negc2c"])
        if WITH_BIAS[1]:
            im["brow"] = w["b1row"].astype(BF16)
        in_maps.append(im)
    r1 = _run(nc1, in_maps)
    xp2_full = scatter_back(plan, [r1.results[c]["xp_out"] for c in range(NCORES)],
                            256, BF16)
    a2_full = scatter_back(plan, [r1.results[c]["a_out"] for c in range(NCORES)],
                           16, BF16)

    # ---- launch 2 (+ pool partials) ----
    in_maps = []
    for c in range(NCORES):
        cc = plan["cores"][c]
        T2 = build_T(plan, c, a2_full, rA.results[c]["el_out"],
                     rA.results[c]["elloop_out"], xp2_full, 256, 2)
        im = dict(T=T2, M=cc["mask"], G=cc["gmask"])
        if WITH_BIAS[2]:
            im["brow"] = w["b2row"].astype(BF16)
        in_maps.append(im)
    r2 = _run(nc2, in_maps)

    # ---- launch E (combine + final linear) ----
    pp = np.concatenate([np.asarray(r2.results[c]["pool_out"], dtype=np.float32)
                         for c in range(NCORES)], 0)
    in_maps = [dict(pp=pp, rcpc=plan["rcp_cnt"][:, None].astype(np.float32),
                    WcT=w["WcT"], bcrow=w["bcrow"])] * NCORES
    rE = _run(ncE, in_maps)
    return np.asarray(rE.results[0]["out"], dtype=np.float32)
